# revision 38
# baseline (speedup 1.0000x reference)
"""Cross-attention kernel for Trainium2, SPMD across 8 NeuronCores.

Problem: B=4, N=M=2048, QD=1024, CD=768, H=8, DH=64, INNER=512 (f32).
  q = x @ Wq; k = ctx @ Wk; v = ctx @ Wv
  out = softmax(q k^T / sqrt(DH)) v @ Wo + bo

Sharding: batch x query-halves -> 8 shards. Core c handles batch c//2,
query rows (c%2)*1024:(c%2+1)*1024, with that batch's full context.
Each core computes a disjoint (1024, 1024) slice of the output; no
cross-core communication. Weights replicated (bf16, matching on-device
compute precision).

Per-core dataflow (all inputs pre-swizzled on host so each tensor is a
single contiguous [128, X] DMA; every matmul contraction dim lands on
SBUF partitions; zero on-device transposes):
  qT = scale * Wq^T @ xT          [INNER, n]   bf16
  kT = Wk^T @ ctxT                [INNER, m]   bf16
  v  = ctxT^T @ Wv                [m, INNER]   bf16
  sT_h = kT_h^T q_h               [m, n] via K=64 matmuls, row-tiled
     pairs (row_grp 0/64). Score psum is DT-MAJOR: psum tile `dt`
     holds both heads of m-tile t=2p+dt side by side, so the two
     row-tiled head matmuls gate on ONE exp (same bank) from the
     previous slot and launch concurrently; hh-major banking made the
     second head matmul wait on the previous slot's second exp (~1.1us
     later), serializing the pair and exposing its LDWEIGHTS.
  E = exp(sT) on ScalarE, [128,1024] (one psum dt-bank) per op; the
     dt0 exp is emitted between the two score pairs so ACT starts two
     matmuls earlier. No max subtraction (|s| < 3 here).
  [O'_h; r_h] = [v_h | 1]^T @ E_h: v tiles pre-padded [v_h | ones] so
     one full-width matmul per (head, m-tile) yields A.V (psum rows
     0:64) and softmax denominators broadcast (rows 64:128).
  O = O' * (1/r)                  [128, n] bf16, packed by INNER tile
  out = sum_j O_j^T @ Wo_j + bo   (K=128 matmuls; bias added in the
     psum-drain DVE op against a pre-broadcast bias tile)

Schedule: flat software pipeline over 64 double-slots (8 groups x 8
m-tile-pairs); A.V lags LAG_D slots behind scores; projection emits are
slotted as filler ordered by DMA arrival and first consumption. Input
DMAs ride one queue in strict priority order (sequential order IS the
priority); output DMAs rotate across sync/gpsimd/scalar queues so
descriptor generation overlaps. The ones-padding memsets run on the
(otherwise idle) GPSIMD engine; v psum drains are batched one strided
copy per m-tile. The last 8 slots' dt1 exps run on the DVE (integer
exp) so ScalarE's backlog never gates the A.V drain, and the tail
output blocks pre-accumulate their j<=2 final-projection partials in
carved psum while the last group's norm chain runs.

History: 241.4us baseline -> 207.3us (dt-major score psum restoring
row-tiled pair concurrency; host-preswizzled single-transfer DMAs;
gpsimd memsets; batched v drains; end-exp engine split; partial-j tail
finals; norm(7) repack on ScalarE; progressive k-tile sub-DMAs for the
first ctx quarter / x half so the prologue starts on partial arrival
and the PE clock gate stays open through the DMA window).
"""

import numpy as np

B, N, M = 4, 2048, 2048
QD, CD = 1024, 768
H, DH = 8, 64
INNER = H * DH  # 512
NS = 1024  # query rows per core
SCALE = DH ** -0.5

_CACHED_NC = None


def build_nc():
    import concourse.bacc as bacc
    import concourse.mybir as mybir
    import concourse.tile as tile

    f32 = mybir.dt.float32
    bf16 = mybir.dt.bfloat16
    FT = mybir.ActivationFunctionType
    AluOp = mybir.AluOpType

    KQ = QD // 128     # 8 k-tiles, q projection
    KC = CD // 128     # 6 k-tiles, k/v projections
    NI = INNER // 128  # 4 partition tiles of INNER (head pairs)
    MT = M // 128      # 16 context m-tiles
    NB = NS // 512     # 2 query blocks
    LAG_D = 4          # A.V lag (double-slots): far enough that exp(s)
                       # has drained before av(s) needs E(s), short
                       # enough that the end-of-loop A.V drain is small

    nc = bacc.Bacc(None)
    # host pre-swizzled: every tensor one contiguous [128, X] transfer.
    # The two tensors gating the first compute (ctx quarter 0, x half 0)
    # are further split by k-tiles into separate tiles/DMAs so their
    # completion sems fire progressively: the prologue chains start on
    # the first sub-arrival, which also keeps the PE clock gate open
    # through the DMA window (whole-tensor sems left the PE idle ~2.5us
    # and the first chains ran at the 1.2GHz mid p-state).
    x0_d = [nc.dram_tensor(f"x0{p}", (128, 4 * 512), bf16,
                           kind="ExternalInput") for p in ("a", "b")]
    x1_d = nc.dram_tensor("x1", (128, KQ * 512), bf16,
                          kind="ExternalInput")
    c0_d = [nc.dram_tensor(f"c0{p}", (128, 3 * 512), bf16,
                           kind="ExternalInput") for p in ("a", "b")]
    c1_d = [nc.dram_tensor(f"c1{p}", (128, 3 * 512), bf16,
                           kind="ExternalInput") for p in ("a", "b")]
    ctx_d = [nc.dram_tensor(f"ctx{i}", (128, KC * 512), bf16,
                            kind="ExternalInput") for i in range(2, 4)]
    wq_d = [nc.dram_tensor(f"wq{p}", (128, 4 * INNER), bf16,
                           kind="ExternalInput") for p in ("a", "b")]
    Wk_d = nc.dram_tensor("Wk", (128, KC * INNER), bf16,
                          kind="ExternalInput")
    Wv_d = nc.dram_tensor("Wv", (128, KC * INNER), bf16,
                          kind="ExternalInput")
    Wo_d = nc.dram_tensor("Wo", (128, NI * QD), bf16,
                          kind="ExternalInput")
    bo_d = nc.dram_tensor("bo", (1, QD), bf16, kind="ExternalInput")
    out_d = nc.dram_tensor("out", (NS, QD), bf16, kind="ExternalOutput")

    with tile.TileContext(nc) as tc:
        with (
            tc.tile_pool(name="w", bufs=1) as wp,
            tc.tile_pool(name="a", bufs=1) as ap,
            tc.tile_pool(name="e", bufs=16) as ep,
            tc.tile_pool(name="s", bufs=1) as sp,
            tc.tile_pool(name="o", bufs=4) as op_,
            tc.tile_pool(name="ps", bufs=2, space="PSUM") as pp,
            tc.tile_pool(name="po", bufs=1, space="PSUM") as ppo,
            tc.tile_pool(name="pss", bufs=2, space="PSUM") as pps,
        ):
            def wtile(cols, name):
                return wp.tile([128, cols], bf16, tag=name, name=name)

            wq_sbs = [wtile(4 * INNER, f"wq{p}") for p in range(2)]
            x0_sb = [wtile(4 * 512, f"x0{p}") for p in range(2)]
            x1_sb = wtile(KQ * 512, "xs1")
            wk_sb = wtile(KC * INNER, "wk")
            c0_sb = [wtile(3 * 512, f"c0{p}") for p in range(2)]
            c1_sb = [wtile(3 * 512, f"c1{p}") for p in range(2)]
            ctx_q = [None, None] + [wtile(KC * 512, f"cs{i}")
                                    for i in range(2, 4)]
            wv_sb = wtile(KC * INNER, "wv")
            wo_sb = wtile(NI * QD, "wo")
            bo_sb = wp.tile([1, QD], bf16, tag="bo", name="bo_sb")
            bo_bc = wp.tile([128, QD], bf16, tag="bob", name="bo_bc")

            # one DMA queue, sequential order = priority order, tuned to
            # first-consumption times: wk+ctx0 feed the prologue kT,
            # wq+x0 the first qT, wv the early v emits; the rest are
            # consumed by filler slots well after they land.
            # one DMA queue: sequential order IS the priority order (a
            # second queue -- even row-splitting single tensors -- only
            # halves per-queue throughput and delays the critical first
            # tensors), ordered by first consumption
            for t, dram in (
                (wk_sb, Wk_d), (c0_sb[0], c0_d[0]), (c0_sb[1], c0_d[1]),
                (wq_sbs[0], wq_d[0]), (x0_sb[0], x0_d[0]),
                (wq_sbs[1], wq_d[1]), (x0_sb[1], x0_d[1]),
                (c1_sb[0], c1_d[0]), (c1_sb[1], c1_d[1]), (wv_sb, Wv_d),
                (ctx_q[2], ctx_d[0]), (ctx_q[3], ctx_d[1]),
                (x1_sb, x1_d), (wo_sb, Wo_d), (bo_sb, bo_d),
            ):
                nc.sync.dma_start(t[:], dram[:])
            nc.gpsimd.partition_broadcast(bo_bc[:], bo_sb[:], channels=128)
            ones_m = wp.tile([128, 64], bf16, tag="onm", name="ones_m")
            nc.vector.memset(ones_m[:], 1.0)

            # HAM warmup: PE clock gate opens after ~3.4us of sustained
            # matmul activity; dummy matmuls bridge the input-DMA waits.
            warm_ps = pps.tile([128, 1024], f32, tag="pss", name="warm")

            def warmup(n):
                for _ in range(n):
                    nc.tensor.matmul(warm_ps[0:64, 0:64], ones_m[:],
                                     ones_m[:], start=True, stop=True)

            def wqs(k, j):
                t, kk = wq_sbs[k // 4], k % 4
                return t[:, kk * INNER + j * 128:kk * INNER + (j + 1) * 128]

            def xts(k, nb):
                if nb == 1:
                    return x1_sb[:, k * 512:(k + 1) * 512]
                t = x0_sb[k // 4]
                return t[:, (k % 4) * 512:(k % 4 + 1) * 512]

            def wks(k, j):
                return wk_sb[:, k * INNER + j * 128:k * INNER + (j + 1) * 128]

            def ctxs(k, lo, sz):
                q, l2 = divmod(lo, 512)
                if q <= 1:
                    t = (c0_sb, c1_sb)[q][k // 3]
                    kk = k % 3
                    return t[:, kk * 512 + l2:kk * 512 + l2 + sz]
                return ctx_q[q][:, k * 512 + l2:k * 512 + l2 + sz]

            def wvs(k):
                return wv_sb[:, k * INNER:(k + 1) * INNER]

            def wos(j, qb):
                return wo_sb[:, j * QD + qb * 512:j * QD + (qb + 1) * 512]

            qT = [[ap.tile([128, 512], bf16, tag=f"qT{j}_{nb}",
                           name=f"qT{j}_{nb}") for nb in range(NB)]
                  for j in range(NI)]
            kT = [[ap.tile([128, 512], bf16, tag=f"kT{j}_{mb}",
                           name=f"kT{j}_{mb}") for mb in range(4)]
                  for j in range(NI)]
            # v tiles hold [v_h | ones] per head (cols h*128..h*128+64 =
            # V projection, +64..+128 = 1.0) so the fused A.V matmul
            # yields denominators from the same E stream.
            v = [ap.tile([128, 2 * INNER], bf16, tag=f"v{t}", name=f"v{t}")
                 for t in range(MT)]
            for t in range(MT):
                nc.gpsimd.memset(
                    v[t][:].rearrange("p (h c) -> p h c", c=128)[:, :, 64:128],
                    1.0)
            On = [ap.tile([128, NS], bf16, tag=f"On{j}", name=f"On{j}")
                  for j in range(NI)]

            def emit_qT(j, nb):
                ps = pp.tile([128, 512], f32, tag="pp", name="pp")
                for k in range(KQ):
                    nc.tensor.matmul(ps[:], wqs(k, j), xts(k, nb),
                                     start=(k == 0), stop=(k == KQ - 1))
                nc.vector.tensor_scalar_mul(qT[j][nb][:], ps[:], SCALE)

            def emit_kT(j, mb):
                ps = pp.tile([128, 512], f32, tag="pp", name="pp")
                for k in range(KC):
                    nc.tensor.matmul(ps[:], wks(k, j), ctxs(k, mb * 512, 512),
                                     start=(k == 0), stop=(k == KC - 1))
                nc.vector.tensor_copy(kT[j][mb][:], ps[:])

            def emit_v(t):
                ps = pp.tile([128, 512], f32, tag="pp", name="pp")
                for k in range(KC):
                    nc.tensor.matmul(ps[:], ctxs(k, t * 128, 128), wvs(k),
                                     start=(k == 0), stop=(k == KC - 1))
                # one strided drain for all 8 heads
                nc.vector.tensor_copy(
                    v[t][:].rearrange("p (h c) -> p h c", c=128)[:, :, 0:64],
                    ps[:].rearrange("p (h d) -> p h d", d=64))

            # prologue feeds group 0 slot 0; later kT(0,mb) are fillers
            # (their ctx quarters arrive progressively).
            # group gi = nb*4+j: kT(j,mb) first used at slot 8j+2mb (nb0)
            # / 8(4+j)+2mb (nb1); qT(j,nb) at slot 8(4nb+j).
            filler = {0: (emit_kT, (0, 1)), 1: (emit_qT, (1, 0)),
                      3: (emit_kT, (0, 2)), 4: (emit_kT, (1, 0)),
                      5: (emit_kT, (0, 3)), 6: (emit_kT, (1, 1)),
                      7: (emit_qT, (2, 0)), 8: (emit_kT, (1, 2)),
                      9: (emit_kT, (1, 3)), 10: (emit_kT, (2, 0)),
                      11: (emit_kT, (2, 1)), 12: (emit_kT, (2, 2)),
                      13: (emit_kT, (2, 3)), 14: (emit_qT, (3, 0)),
                      15: (emit_kT, (3, 0)), 16: (emit_kT, (3, 1)),
                      17: (emit_kT, (3, 2)), 18: (emit_kT, (3, 3)),
                      20: (emit_qT, (0, 1)), 21: (emit_qT, (1, 1)),
                      23: (emit_qT, (2, 1)), 25: (emit_qT, (3, 1))}
            vslot = {}
            for t in range(MT):
                vslot.setdefault(min(t // 2 + 2, 9), []).append(t)

            groups = [(j, nb) for nb in range(NB) for j in range(NI)]
            NDS = len(groups) * (MT // 2)  # 64 double-slots
            E = {}    # double-slot -> (E_dt0, E_dt1) [128, 1024] bf16
                      # E_dt[:, hh*512:(hh+1)*512] = head hh, m-tile 2p+dt
            PO = {}   # group idx -> [128, 1024] psum accumulator

            # Last slots split exp across engines: dt0 on ScalarE, dt1 on
            # the DVE via the Schraudolph integer trick (bf16 bits =
            # round(s*128/ln2 + 16249), one tensor_scalar, ~1.8% rms on
            # 6% of tiles -> ~0.45% output contribution). ScalarE's exp
            # backlog otherwise gates the end-of-kernel A.V drain; an
            # all-DVE tail just moved the backlog to the DVE.
            EXP_OFF = set(range(56, 64))
            EXP_A = 128 / float(np.log(2.0))
            EXP_B = 16249.0

            def emit_scores(s):
                j, nb = groups[s // (MT // 2)]
                p = s % (MT // 2)
                # dt-major psum: bank dt holds both heads of m-tile 2p+dt
                psab = [pps.tile([128, 1024], f32, tag="pss", name="pss")
                        for _ in range(2)]
                es = []
                for dt in range(2):
                    t = 2 * p + dt
                    for hh in range(2):
                        nc.tensor.matmul(
                            psab[dt][:, hh * 512:(hh + 1) * 512],
                            kT[j][t // 4][hh * 64:(hh + 1) * 64,
                                          (t % 4) * 128:(t % 4 + 1) * 128],
                            qT[j][nb][hh * 64:(hh + 1) * 64, :],
                            start=True, stop=True)
                    e = ep.tile([128, 1024], bf16, tag="E", name="E")
                    if s in EXP_OFF and dt == 1:
                        nc.vector.tensor_scalar(
                            e[:].bitcast(mybir.dt.int16), psab[dt][:],
                            EXP_A, EXP_B, AluOp.mult, AluOp.add)
                    else:
                        nc.scalar.activation(e[:], psab[dt][:], FT.Exp)
                    es.append(e)
                E[s] = es

            def emit_av(s):
                gi = s // (MT // 2)
                j, nb = groups[gi]
                p = s % (MT // 2)
                if p == 0:
                    PO[gi] = ppo.tile([128, 1024], f32, tag="po", name="po")
                po = PO[gi]
                for dt in range(2):
                    t = 2 * p + dt
                    st, sp_ = (t == 0), (t == MT - 1)
                    for hh in range(2):
                        h = 2 * j + hh
                        nc.tensor.matmul(
                            po[:, hh * 512:(hh + 1) * 512],
                            v[t][:, h * 128:(h + 1) * 128],
                            E[s][dt][:, hh * 512:(hh + 1) * 512],
                            start=st, stop=sp_, skip_group_check=True)
                del E[s]

            def emit_norm(gi, last=False):
                j, nb = groups[gi]
                po = PO[gi]
                # one full-range psum->sbuf copy frees the single A.V
                # psum buffer fastest (next group's first A.V waits on
                # it); repack [O'; r] into aligned operands -- DVE psum
                # reads must keep the same partition range as the
                # output, and reciprocal_approx_fast (custom DVE ucode)
                # needs an aligned sbuf input. For the LAST group this
                # serial chain is the end-of-kernel critical path, so
                # the partition-aligned copies ride the (by then idle)
                # ScalarE, halving the repack latency.
                act = nc.scalar.copy if last else nc.vector.tensor_copy
                rs = sp.tile([128, 1024], f32, tag="rs", name="rs")
                act(rs[:], po[:])
                rr = sp.tile([128, 512], f32, tag="rr", name="rr")
                op2 = sp.tile([128, 512], f32, tag="op2", name="op2")
                act(rr[64:128, :], rs[64:128, 512:1024])
                act(op2[0:64, :], rs[0:64, 0:512])
                nc.vector.tensor_copy(rr[0:64, :], rs[64:128, 0:512])
                nc.vector.tensor_copy(op2[64:128, :], rs[0:64, 512:1024])
                rb = sp.tile([128, 512], f32, tag="rb", name="rb")
                nc.vector.reciprocal_approx_fast(rb[:], rr[:])
                nc.vector.tensor_tensor(
                    On[j][:, nb * 512:(nb + 1) * 512], op2[:], rb[:],
                    op=AluOp.mult)
                del PO[gi]

            OT = {}   # nt -> [128, 1024] bf16 output row-block
            DMA_ENGS = [nc.sync, nc.gpsimd]

            def emit_final(nt, qb, last=False):
                pf = pp.tile([128, 512], f32, tag="pp", name="pf")
                for j in range(NI):
                    nc.tensor.matmul(
                        pf[:], On[j][:, nt * 128:(nt + 1) * 128],
                        wos(j, qb), start=(j == 0), stop=(j == NI - 1))
                if qb == 0:
                    OT[nt] = op_.tile([128, QD], bf16, tag="ot",
                                      name=f"ot{nt}")
                ot = OT[nt]
                nc.vector.tensor_tensor(
                    ot[:, qb * 512:(qb + 1) * 512], pf[:],
                    bo_bc[:, qb * 512:(qb + 1) * 512], op=AluOp.add)
                if qb == 1:
                    # rotate descriptor generation across idle queues so
                    # the end-of-kernel DMA chains run in parallel
                    eng = nc.scalar if last else DMA_ENGS[nt % 2]
                    eng.dma_start(out_d[nt * 128:(nt + 1) * 128, :], ot[:])
                    del OT[nt]

            FIN = {37: (0, 0), 39: (0, 1), 41: (1, 0), 43: (1, 1),
                   45: (2, 0), 47: (2, 1), 49: (3, 0), 51: (3, 1)}

            # prologue: sized to the single-queue DMA arrival times
            # (wk ~4us, ctx0 ~9us, wq ~12us, x0 ~15us)
            warmup(46)
            emit_kT(0, 0)
            warmup(16)
            emit_qT(0, 0)
            warmup(10)

            for s in range(NDS + LAG_D):
                if s < NDS:
                    emit_scores(s)
                    for t in vslot.get(s, []):
                        emit_v(t)
                    if s in filler:
                        fn, args = filler[s]
                        fn(*args)
                    if s in FIN:
                        emit_final(*FIN[s])
                a = s - LAG_D
                if a >= 0:
                    emit_av(a)
                    if a % (MT // 2) == MT // 2 - 1:
                        gi = a // (MT // 2)
                        emit_norm(gi, last=(gi == len(groups) - 1))

            # tail: nb1 row-blocks. The j<=2 partial accumulations sit
            # right behind the last A.V in the PE queue, so they fill
            # the norm(7) window with useful work; only one j=3 matmul
            # per block remains on the post-norm critical path. Psum is
            # carved from the score pool (free once the last exps have
            # read it) and the proj pool; the last block reuses the A.V
            # bank freed by norm(7)'s drain.
            def tail_mm(pf, nt, qb, jlo, jhi, start, stop):
                for j in range(jlo, jhi):
                    nc.tensor.matmul(
                        pf, On[j][:, nt * 128:(nt + 1) * 128],
                        wos(j, qb), start=(start and j == jlo),
                        stop=(stop and j == jhi - 1),
                        skip_group_check=True)

            pf45 = [pps.tile([128, 1024], f32, tag="pss", name=f"pf{nt}")
                    for nt in (4, 5)]
            pf6 = [pp.tile([128, 512], f32, tag="pp", name=f"pf6_{qb}")
                   for qb in range(2)]
            for i, nt in enumerate((4, 5)):
                for qb in range(2):
                    tail_mm(pf45[i][:, qb * 512:(qb + 1) * 512],
                            nt, qb, 0, 3, True, False)
            for qb in range(2):
                tail_mm(pf6[qb][:], 6, qb, 0, 3, True, False)
            for i, nt in enumerate((4, 5)):
                ot = op_.tile([128, QD], bf16, tag="ot", name=f"ot{nt}")
                for qb in range(2):
                    tail_mm(pf45[i][:, qb * 512:(qb + 1) * 512],
                            nt, qb, 3, 4, False, True)
                nc.vector.tensor_tensor(ot[:], pf45[i][:], bo_bc[:],
                                        op=AluOp.add)
                DMA_ENGS[nt % 2].dma_start(
                    out_d[nt * 128:(nt + 1) * 128, :], ot[:])
            ot6 = op_.tile([128, QD], bf16, tag="ot", name="ot6")
            for qb in range(2):
                tail_mm(pf6[qb][:], 6, qb, 3, 4, False, True)
                nc.vector.tensor_tensor(
                    ot6[:, qb * 512:(qb + 1) * 512], pf6[qb][:],
                    bo_bc[:, qb * 512:(qb + 1) * 512], op=AluOp.add)
            nc.sync.dma_start(out_d[6 * 128:7 * 128, :], ot6[:])
            pf7 = ppo.tile([128, 1024], f32, tag="po", name="pf7")
            ot7 = op_.tile([128, QD], bf16, tag="ot", name="ot7")
            for qb in range(2):
                tail_mm(pf7[:, qb * 512:(qb + 1) * 512],
                        7, qb, 0, 4, True, True)
                nc.vector.tensor_tensor(
                    ot7[:, qb * 512:(qb + 1) * 512],
                    pf7[:, qb * 512:(qb + 1) * 512],
                    bo_bc[:, qb * 512:(qb + 1) * 512], op=AluOp.add)
                nc.scalar.dma_start(
                    out_d[7 * 128:8 * 128, qb * 512:(qb + 1) * 512],
                    ot7[:, qb * 512:(qb + 1) * 512])
    nc.compile()
    return nc


def _get_nc():
    global _CACHED_NC
    if _CACHED_NC is None:
        _CACHED_NC = build_nc()
    return _CACHED_NC


def _sw(a, k):
    """[k*128, C] -> [128, k*C] partition-swizzle (k-tile major cols)."""
    c = a.shape[1]
    return np.ascontiguousarray(
        a.reshape(k, 128, c).transpose(1, 0, 2).reshape(128, k * c))


def _shard_inputs(x, context, Wq, Wk, Wv, Wo, bo):
    import ml_dtypes
    bf = ml_dtypes.bfloat16
    KQ, KC, NI = QD // 128, CD // 128, INNER // 128
    Wqs = _sw(np.asarray(Wq).astype(bf), KQ)
    wqa = np.ascontiguousarray(Wqs[:, 0:4 * INNER])
    wqb = np.ascontiguousarray(Wqs[:, 4 * INNER:])
    Wks = _sw(np.asarray(Wk).astype(bf), KC)
    Wvs = _sw(np.asarray(Wv).astype(bf), KC)
    Wos = _sw(np.asarray(Wo).astype(bf), NI)
    bo2 = np.ascontiguousarray(np.asarray(bo).astype(bf).reshape(1, QD))
    in_maps = []
    for c in range(8):
        b, q = divmod(c, 2)
        xT = np.asarray(x[b, q * NS:(q + 1) * NS, :]).astype(bf).T
        xTs = xT.reshape(KQ, 128, NS).transpose(1, 0, 2)  # [128, KQ, NS]
        cT = np.asarray(context[b]).astype(bf).T
        cTs = cT.reshape(KC, 128, M).transpose(1, 0, 2)   # [128, KC, M]
        im = {"wqa": wqa, "wqb": wqb, "Wk": Wks, "Wv": Wvs, "Wo": Wos,
              "bo": bo2}
        x0 = xTs[:, :, 0:512]
        im["x0a"] = np.ascontiguousarray(x0[:, 0:4].reshape(128, -1))
        im["x0b"] = np.ascontiguousarray(x0[:, 4:8].reshape(128, -1))
        im["x1"] = np.ascontiguousarray(
            xTs[:, :, 512:1024].reshape(128, -1))
        for q, nm in ((0, "c0"), (1, "c1")):
            cq = cTs[:, :, q * 512:(q + 1) * 512]
            im[nm + "a"] = np.ascontiguousarray(cq[:, 0:3].reshape(128, -1))
            im[nm + "b"] = np.ascontiguousarray(cq[:, 3:6].reshape(128, -1))
        for i in range(2, 4):
            im[f"ctx{i}"] = np.ascontiguousarray(
                cTs[:, :, i * 512:(i + 1) * 512].reshape(128, -1))
        in_maps.append(im)
    return in_maps


def kernel(x, context, Wq, Wk, Wv, Wo, bo, _trace=False):
    from concourse.bass_utils import run_bass_kernel_spmd

    nc = _get_nc()
    in_maps = _shard_inputs(x, context, Wq, Wk, Wv, Wo, bo)
    res = run_bass_kernel_spmd(nc, in_maps, core_ids=list(range(8)),
                               trace=_trace)
    out = np.empty((B, N, QD), np.float32)
    for c in range(8):
        b, q = divmod(c, 2)
        out[b, q * NS:(q + 1) * NS, :] = res.results[c]["out"].astype(
            np.float32)
    if _trace:
        kernel._last_result = res
    return out


# revision 39
# speedup vs baseline: 1.0159x; 1.0159x over previous
"""Cross-attention kernel for Trainium2, SPMD across 8 NeuronCores.

Problem: B=4, N=M=2048, QD=1024, CD=768, H=8, DH=64, INNER=512 (f32).
  q = x @ Wq; k = ctx @ Wk; v = ctx @ Wv
  out = softmax(q k^T / sqrt(DH)) v @ Wo + bo

Sharding: batch x query-halves -> 8 shards. Core c handles batch c//2,
query rows (c%2)*1024:(c%2+1)*1024, with that batch's full context.
Each core computes a disjoint (1024, 1024) slice of the output; no
cross-core communication. Weights replicated (bf16, matching on-device
compute precision).

Per-core dataflow (all inputs pre-swizzled on host so each tensor is a
single contiguous [128, X] DMA; every matmul contraction dim lands on
SBUF partitions; zero on-device transposes):
  qT = scale * Wq^T @ xT          [INNER, n]   bf16
  kT = Wk^T @ ctxT                [INNER, m]   bf16
  v  = ctxT^T @ Wv                [m, INNER]   bf16
  sT_h = kT_h^T q_h               [m, n] via K=64 matmuls, row-tiled
     pairs (row_grp 0/64). Score psum is DT-MAJOR: psum tile `dt`
     holds both heads of m-tile t=2p+dt side by side, so the two
     row-tiled head matmuls gate on ONE exp (same bank) from the
     previous slot and launch concurrently; hh-major banking made the
     second head matmul wait on the previous slot's second exp (~1.1us
     later), serializing the pair and exposing its LDWEIGHTS.
  E = exp(sT) on ScalarE, [128,1024] (one psum dt-bank) per op; the
     dt0 exp is emitted between the two score pairs so ACT starts two
     matmuls earlier. No max subtraction (|s| < 3 here).
  [O'_h; r_h] = [v_h | 1]^T @ E_h: v tiles pre-padded [v_h | ones] so
     one full-width matmul per (head, m-tile) yields A.V (psum rows
     0:64) and softmax denominators broadcast (rows 64:128).
  O = O' * (1/r)                  [128, n] bf16, packed by INNER tile
  out = sum_j O_j^T @ Wo_j + bo   (K=128 matmuls; bias added in the
     psum-drain DVE op against a pre-broadcast bias tile)

Schedule: flat software pipeline over 64 double-slots (8 groups x 8
m-tile-pairs); A.V lags LAG_D slots behind scores; projection emits are
slotted as filler ordered by DMA arrival and first consumption. Input
DMAs ride one queue in strict priority order (sequential order IS the
priority); output DMAs rotate across sync/gpsimd/scalar queues so
descriptor generation overlaps. The ones-padding memsets run on the
(otherwise idle) GPSIMD engine; v psum drains are batched one strided
copy per m-tile. The last 8 slots' dt1 exps run on the DVE (integer
exp) so ScalarE's backlog never gates the A.V drain, and the tail
output blocks pre-accumulate their j<=2 final-projection partials in
carved psum while the last group's norm chain runs.

History: 241.4us baseline -> 207.3us (dt-major score psum restoring
row-tiled pair concurrency; host-preswizzled single-transfer DMAs;
gpsimd memsets; batched v drains; end-exp engine split; partial-j tail
finals; norm(7) repack on ScalarE; progressive k-tile sub-DMAs for the
first ctx quarter / x half so the prologue starts on partial arrival
and the PE clock gate stays open through the DMA window).
"""

import numpy as np

B, N, M = 4, 2048, 2048
QD, CD = 1024, 768
H, DH = 8, 64
INNER = H * DH  # 512
NS = 1024  # query rows per core
SCALE = DH ** -0.5

_CACHED_NC = None


def build_nc():
    import concourse.bacc as bacc
    import concourse.mybir as mybir
    import concourse.tile as tile

    f32 = mybir.dt.float32
    bf16 = mybir.dt.bfloat16
    FT = mybir.ActivationFunctionType
    AluOp = mybir.AluOpType

    KQ = QD // 128     # 8 k-tiles, q projection
    KC = CD // 128     # 6 k-tiles, k/v projections
    NI = INNER // 128  # 4 partition tiles of INNER (head pairs)
    MT = M // 128      # 16 context m-tiles
    NB = NS // 512     # 2 query blocks
    LAG_D = 4          # A.V lag (double-slots): far enough that exp(s)
                       # has drained before av(s) needs E(s), short
                       # enough that the end-of-loop A.V drain is small

    nc = bacc.Bacc(None)
    # host pre-swizzled: every tensor one contiguous [128, X] transfer.
    # The two tensors gating the first compute (ctx quarter 0, x half 0)
    # are further split by k-tiles into separate tiles/DMAs so their
    # completion sems fire progressively: the prologue chains start on
    # the first sub-arrival, which also keeps the PE clock gate open
    # through the DMA window (whole-tensor sems left the PE idle ~2.5us
    # and the first chains ran at the 1.2GHz mid p-state).
    x0_d = [nc.dram_tensor(f"x0{p}", (128, 4 * 512), bf16,
                           kind="ExternalInput") for p in ("a", "b")]
    x1_d = nc.dram_tensor("x1", (128, KQ * 512), bf16,
                          kind="ExternalInput")
    c0_d = [nc.dram_tensor(f"c0{p}", (128, 3 * 512), bf16,
                           kind="ExternalInput") for p in ("a", "b")]
    ctx_d = [nc.dram_tensor(f"ctx{i}", (128, KC * 512), bf16,
                            kind="ExternalInput") for i in range(1, 4)]
    Wq_d = nc.dram_tensor("Wq", (128, KQ * INNER), bf16,
                          kind="ExternalInput")
    Wk_d = nc.dram_tensor("Wk", (128, KC * INNER), bf16,
                          kind="ExternalInput")
    Wv_d = nc.dram_tensor("Wv", (128, KC * INNER), bf16,
                          kind="ExternalInput")
    Wo_d = nc.dram_tensor("Wo", (128, NI * QD), bf16,
                          kind="ExternalInput")
    bo_d = nc.dram_tensor("bo", (1, QD), bf16, kind="ExternalInput")
    out_d = nc.dram_tensor("out", (NS, QD), bf16, kind="ExternalOutput")

    with tile.TileContext(nc) as tc:
        with (
            tc.tile_pool(name="w", bufs=1) as wp,
            tc.tile_pool(name="a", bufs=1) as ap,
            tc.tile_pool(name="e", bufs=16) as ep,
            tc.tile_pool(name="s", bufs=1) as sp,
            tc.tile_pool(name="o", bufs=4) as op_,
            tc.tile_pool(name="ps", bufs=2, space="PSUM") as pp,
            tc.tile_pool(name="po", bufs=1, space="PSUM") as ppo,
            tc.tile_pool(name="pss", bufs=2, space="PSUM") as pps,
        ):
            def wtile(cols, name):
                return wp.tile([128, cols], bf16, tag=name, name=name)

            wq_sb = wtile(KQ * INNER, "wq")
            x0_sb = [wtile(4 * 512, f"x0{p}") for p in range(2)]
            x1_sb = wtile(KQ * 512, "xs1")
            wk_sb = wtile(KC * INNER, "wk")
            c0_sb = [wtile(3 * 512, f"c0{p}") for p in range(2)]
            ctx_q = [None] + [wtile(KC * 512, f"cs{i}") for i in range(1, 4)]
            wv_sb = wtile(KC * INNER, "wv")
            wo_sb = wtile(NI * QD, "wo")
            bo_sb = wp.tile([1, QD], bf16, tag="bo", name="bo_sb")
            bo_bc = wp.tile([128, QD], bf16, tag="bob", name="bo_bc")

            # one DMA queue, sequential order = priority order, tuned to
            # first-consumption times: wk+ctx0 feed the prologue kT,
            # wq+x0 the first qT, wv the early v emits; the rest are
            # consumed by filler slots well after they land.
            # one DMA queue: sequential order IS the priority order (a
            # second queue -- even row-splitting single tensors -- only
            # halves per-queue throughput and delays the critical first
            # tensors), ordered by first consumption
            for t, dram in (
                (wk_sb, Wk_d), (c0_sb[0], c0_d[0]), (c0_sb[1], c0_d[1]),
                (wq_sb, Wq_d), (x0_sb[0], x0_d[0]), (x0_sb[1], x0_d[1]),
                (ctx_q[1], ctx_d[0]), (wv_sb, Wv_d),
                (ctx_q[2], ctx_d[1]), (ctx_q[3], ctx_d[2]),
                (x1_sb, x1_d), (wo_sb, Wo_d), (bo_sb, bo_d),
            ):
                nc.sync.dma_start(t[:], dram[:])
            nc.gpsimd.partition_broadcast(bo_bc[:], bo_sb[:], channels=128)
            ones_m = wp.tile([128, 64], bf16, tag="onm", name="ones_m")
            nc.vector.memset(ones_m[:], 1.0)

            # HAM warmup: PE clock gate opens after ~3.4us of sustained
            # matmul activity; dummy matmuls bridge the input-DMA waits.
            warm_ps = pps.tile([128, 1024], f32, tag="pss", name="warm")

            def warmup(n):
                for _ in range(n):
                    nc.tensor.matmul(warm_ps[0:64, 0:64], ones_m[:],
                                     ones_m[:], start=True, stop=True)

            def wqs(k, j):
                return wq_sb[:, k * INNER + j * 128:k * INNER + (j + 1) * 128]

            def xts(k, nb):
                if nb == 1:
                    return x1_sb[:, k * 512:(k + 1) * 512]
                t = x0_sb[k // 4]
                return t[:, (k % 4) * 512:(k % 4 + 1) * 512]

            def wks(k, j):
                return wk_sb[:, k * INNER + j * 128:k * INNER + (j + 1) * 128]

            def ctxs(k, lo, sz):
                q, l2 = divmod(lo, 512)
                if q == 0:
                    t, kk = c0_sb[k // 3], k % 3
                    return t[:, kk * 512 + l2:kk * 512 + l2 + sz]
                return ctx_q[q][:, k * 512 + l2:k * 512 + l2 + sz]

            def wvs(k):
                return wv_sb[:, k * INNER:(k + 1) * INNER]

            def wos(j, qb):
                return wo_sb[:, j * QD + qb * 512:j * QD + (qb + 1) * 512]

            qT = [[ap.tile([128, 512], bf16, tag=f"qT{j}_{nb}",
                           name=f"qT{j}_{nb}") for nb in range(NB)]
                  for j in range(NI)]
            kT = [[ap.tile([128, 512], bf16, tag=f"kT{j}_{mb}",
                           name=f"kT{j}_{mb}") for mb in range(4)]
                  for j in range(NI)]
            # v tiles hold [v_h | ones] per head (cols h*128..h*128+64 =
            # V projection, +64..+128 = 1.0) so the fused A.V matmul
            # yields denominators from the same E stream.
            v = [ap.tile([128, 2 * INNER], bf16, tag=f"v{t}", name=f"v{t}")
                 for t in range(MT)]
            for t in range(MT):
                nc.gpsimd.memset(
                    v[t][:].rearrange("p (h c) -> p h c", c=128)[:, :, 64:128],
                    1.0)
            On = [ap.tile([128, NS], bf16, tag=f"On{j}", name=f"On{j}")
                  for j in range(NI)]

            def emit_qT(j, nb):
                ps = pp.tile([128, 512], f32, tag="pp", name="pp")
                for k in range(KQ):
                    nc.tensor.matmul(ps[:], wqs(k, j), xts(k, nb),
                                     start=(k == 0), stop=(k == KQ - 1))
                nc.vector.tensor_scalar_mul(qT[j][nb][:], ps[:], SCALE)

            def emit_kT(j, mb):
                ps = pp.tile([128, 512], f32, tag="pp", name="pp")
                for k in range(KC):
                    nc.tensor.matmul(ps[:], wks(k, j), ctxs(k, mb * 512, 512),
                                     start=(k == 0), stop=(k == KC - 1))
                nc.vector.tensor_copy(kT[j][mb][:], ps[:])

            def emit_v(t):
                ps = pp.tile([128, 512], f32, tag="pp", name="pp")
                for k in range(KC):
                    nc.tensor.matmul(ps[:], ctxs(k, t * 128, 128), wvs(k),
                                     start=(k == 0), stop=(k == KC - 1))
                # one strided drain for all 8 heads
                nc.vector.tensor_copy(
                    v[t][:].rearrange("p (h c) -> p h c", c=128)[:, :, 0:64],
                    ps[:].rearrange("p (h d) -> p h d", d=64))

            # prologue feeds group 0 slot 0; later kT(0,mb) are fillers
            # (their ctx quarters arrive progressively).
            # group gi = nb*4+j: kT(j,mb) first used at slot 8j+2mb (nb0)
            # / 8(4+j)+2mb (nb1); qT(j,nb) at slot 8(4nb+j).
            filler = {0: (emit_kT, (0, 1)), 1: (emit_qT, (1, 0)),
                      3: (emit_kT, (0, 2)), 4: (emit_kT, (1, 0)),
                      5: (emit_kT, (0, 3)), 6: (emit_kT, (1, 1)),
                      7: (emit_qT, (2, 0)), 8: (emit_kT, (1, 2)),
                      9: (emit_kT, (1, 3)), 10: (emit_kT, (2, 0)),
                      11: (emit_kT, (2, 1)), 12: (emit_kT, (2, 2)),
                      13: (emit_kT, (2, 3)), 14: (emit_qT, (3, 0)),
                      15: (emit_kT, (3, 0)), 16: (emit_kT, (3, 1)),
                      17: (emit_kT, (3, 2)), 18: (emit_kT, (3, 3)),
                      20: (emit_qT, (0, 1)), 21: (emit_qT, (1, 1)),
                      23: (emit_qT, (2, 1)), 25: (emit_qT, (3, 1))}
            vslot = {}
            for t in range(MT):
                vslot.setdefault(min(t // 2 + 2, 9), []).append(t)

            groups = [(j, nb) for nb in range(NB) for j in range(NI)]
            NDS = len(groups) * (MT // 2)  # 64 double-slots
            E = {}    # double-slot -> (E_dt0, E_dt1) [128, 1024] bf16
                      # E_dt[:, hh*512:(hh+1)*512] = head hh, m-tile 2p+dt
            PO = {}   # group idx -> [128, 1024] psum accumulator

            # Last slots split exp across engines: dt0 on ScalarE, dt1 on
            # the DVE via the Schraudolph integer trick (bf16 bits =
            # round(s*128/ln2 + 16249), one tensor_scalar, ~1.8% rms on
            # 6% of tiles -> ~0.45% output contribution). ScalarE's exp
            # backlog otherwise gates the end-of-kernel A.V drain; an
            # all-DVE tail just moved the backlog to the DVE.
            EXP_OFF = set(range(56, 64))
            EXP_A = 128 / float(np.log(2.0))
            EXP_B = 16249.0

            def emit_scores(s):
                j, nb = groups[s // (MT // 2)]
                p = s % (MT // 2)
                # dt-major psum: bank dt holds both heads of m-tile 2p+dt
                psab = [pps.tile([128, 1024], f32, tag="pss", name="pss")
                        for _ in range(2)]
                es = []
                for dt in range(2):
                    t = 2 * p + dt
                    for hh in range(2):
                        nc.tensor.matmul(
                            psab[dt][:, hh * 512:(hh + 1) * 512],
                            kT[j][t // 4][hh * 64:(hh + 1) * 64,
                                          (t % 4) * 128:(t % 4 + 1) * 128],
                            qT[j][nb][hh * 64:(hh + 1) * 64, :],
                            start=True, stop=True)
                    e = ep.tile([128, 1024], bf16, tag="E", name="E")
                    if s in EXP_OFF and dt == 1:
                        nc.vector.tensor_scalar(
                            e[:].bitcast(mybir.dt.int16), psab[dt][:],
                            EXP_A, EXP_B, AluOp.mult, AluOp.add)
                    else:
                        nc.scalar.activation(e[:], psab[dt][:], FT.Exp)
                    es.append(e)
                E[s] = es

            def emit_av(s):
                gi = s // (MT // 2)
                j, nb = groups[gi]
                p = s % (MT // 2)
                if p == 0:
                    PO[gi] = ppo.tile([128, 1024], f32, tag="po", name="po")
                po = PO[gi]
                for dt in range(2):
                    t = 2 * p + dt
                    st, sp_ = (t == 0), (t == MT - 1)
                    for hh in range(2):
                        h = 2 * j + hh
                        nc.tensor.matmul(
                            po[:, hh * 512:(hh + 1) * 512],
                            v[t][:, h * 128:(h + 1) * 128],
                            E[s][dt][:, hh * 512:(hh + 1) * 512],
                            start=st, stop=sp_, skip_group_check=True)
                del E[s]

            def emit_norm(gi, last=False):
                j, nb = groups[gi]
                po = PO[gi]
                # one full-range psum->sbuf copy frees the single A.V
                # psum buffer fastest (next group's first A.V waits on
                # it); repack [O'; r] into aligned operands -- DVE psum
                # reads must keep the same partition range as the
                # output, and reciprocal_approx_fast (custom DVE ucode)
                # needs an aligned sbuf input. For the LAST group this
                # serial chain is the end-of-kernel critical path, so
                # the partition-aligned copies ride the (by then idle)
                # ScalarE, halving the repack latency.
                act = nc.scalar.copy if last else nc.vector.tensor_copy
                rs = sp.tile([128, 1024], f32, tag="rs", name="rs")
                act(rs[:], po[:])
                rr = sp.tile([128, 512], f32, tag="rr", name="rr")
                op2 = sp.tile([128, 512], f32, tag="op2", name="op2")
                act(rr[64:128, :], rs[64:128, 512:1024])
                act(op2[0:64, :], rs[0:64, 0:512])
                nc.vector.tensor_copy(rr[0:64, :], rs[64:128, 0:512])
                nc.vector.tensor_copy(op2[64:128, :], rs[0:64, 512:1024])
                rb = sp.tile([128, 512], f32, tag="rb", name="rb")
                nc.vector.reciprocal_approx_fast(rb[:], rr[:])
                nc.vector.tensor_tensor(
                    On[j][:, nb * 512:(nb + 1) * 512], op2[:], rb[:],
                    op=AluOp.mult)
                del PO[gi]

            OT = {}   # nt -> [128, 1024] bf16 output row-block
            DMA_ENGS = [nc.sync, nc.gpsimd]

            def emit_final(nt, qb, last=False):
                pf = pp.tile([128, 512], f32, tag="pp", name="pf")
                for j in range(NI):
                    nc.tensor.matmul(
                        pf[:], On[j][:, nt * 128:(nt + 1) * 128],
                        wos(j, qb), start=(j == 0), stop=(j == NI - 1))
                if qb == 0:
                    OT[nt] = op_.tile([128, QD], bf16, tag="ot",
                                      name=f"ot{nt}")
                ot = OT[nt]
                nc.vector.tensor_tensor(
                    ot[:, qb * 512:(qb + 1) * 512], pf[:],
                    bo_bc[:, qb * 512:(qb + 1) * 512], op=AluOp.add)
                if qb == 1:
                    # rotate descriptor generation across idle queues so
                    # the end-of-kernel DMA chains run in parallel
                    eng = nc.scalar if last else DMA_ENGS[nt % 2]
                    eng.dma_start(out_d[nt * 128:(nt + 1) * 128, :], ot[:])
                    del OT[nt]

            FIN = {37: (0, 0), 39: (0, 1), 41: (1, 0), 43: (1, 1),
                   45: (2, 0), 47: (2, 1), 49: (3, 0), 51: (3, 1)}

            # prologue: sized to the single-queue DMA arrival times
            # (wk ~4us, ctx0 ~9us, wq ~12us, x0 ~15us)
            warmup(46)
            emit_kT(0, 0)
            warmup(16)
            emit_qT(0, 0)
            warmup(10)

            for s in range(NDS + LAG_D):
                if s < NDS:
                    emit_scores(s)
                    for t in vslot.get(s, []):
                        emit_v(t)
                    if s in filler:
                        fn, args = filler[s]
                        fn(*args)
                    if s in FIN:
                        emit_final(*FIN[s])
                a = s - LAG_D
                if a >= 0:
                    emit_av(a)
                    if a % (MT // 2) == MT // 2 - 1:
                        gi = a // (MT // 2)
                        emit_norm(gi, last=(gi == len(groups) - 1))

            # tail: nb1 row-blocks. The j<=2 partial accumulations sit
            # right behind the last A.V in the PE queue, so they fill
            # the norm(7) window with useful work; only one j=3 matmul
            # per block remains on the post-norm critical path. Psum is
            # carved from the score pool (free once the last exps have
            # read it) and the proj pool; the last block reuses the A.V
            # bank freed by norm(7)'s drain.
            def tail_mm(pf, nt, qb, jlo, jhi, start, stop):
                for j in range(jlo, jhi):
                    nc.tensor.matmul(
                        pf, On[j][:, nt * 128:(nt + 1) * 128],
                        wos(j, qb), start=(start and j == jlo),
                        stop=(stop and j == jhi - 1),
                        skip_group_check=True)

            pf45 = [pps.tile([128, 1024], f32, tag="pss", name=f"pf{nt}")
                    for nt in (4, 5)]
            pf6 = [pp.tile([128, 512], f32, tag="pp", name=f"pf6_{qb}")
                   for qb in range(2)]
            for i, nt in enumerate((4, 5)):
                for qb in range(2):
                    tail_mm(pf45[i][:, qb * 512:(qb + 1) * 512],
                            nt, qb, 0, 3, True, False)
            for qb in range(2):
                tail_mm(pf6[qb][:], 6, qb, 0, 3, True, False)
            for i, nt in enumerate((4, 5)):
                ot = op_.tile([128, QD], bf16, tag="ot", name=f"ot{nt}")
                for qb in range(2):
                    tail_mm(pf45[i][:, qb * 512:(qb + 1) * 512],
                            nt, qb, 3, 4, False, True)
                nc.vector.tensor_tensor(ot[:], pf45[i][:], bo_bc[:],
                                        op=AluOp.add)
                DMA_ENGS[nt % 2].dma_start(
                    out_d[nt * 128:(nt + 1) * 128, :], ot[:])
            ot6 = op_.tile([128, QD], bf16, tag="ot", name="ot6")
            for qb in range(2):
                tail_mm(pf6[qb][:], 6, qb, 3, 4, False, True)
                nc.vector.tensor_tensor(
                    ot6[:, qb * 512:(qb + 1) * 512], pf6[qb][:],
                    bo_bc[:, qb * 512:(qb + 1) * 512], op=AluOp.add)
            nc.sync.dma_start(out_d[6 * 128:7 * 128, :], ot6[:])
            pf7 = ppo.tile([128, 1024], f32, tag="po", name="pf7")
            ot7 = op_.tile([128, QD], bf16, tag="ot", name="ot7")
            for qb in range(2):
                tail_mm(pf7[:, qb * 512:(qb + 1) * 512],
                        7, qb, 0, 4, True, True)
                nc.vector.tensor_tensor(
                    ot7[:, qb * 512:(qb + 1) * 512],
                    pf7[:, qb * 512:(qb + 1) * 512],
                    bo_bc[:, qb * 512:(qb + 1) * 512], op=AluOp.add)
                nc.scalar.dma_start(
                    out_d[7 * 128:8 * 128, qb * 512:(qb + 1) * 512],
                    ot7[:, qb * 512:(qb + 1) * 512])
    nc.compile()
    return nc


def _get_nc():
    global _CACHED_NC
    if _CACHED_NC is None:
        _CACHED_NC = build_nc()
    return _CACHED_NC


def _sw(a, k):
    """[k*128, C] -> [128, k*C] partition-swizzle (k-tile major cols)."""
    c = a.shape[1]
    return np.ascontiguousarray(
        a.reshape(k, 128, c).transpose(1, 0, 2).reshape(128, k * c))


def _shard_inputs(x, context, Wq, Wk, Wv, Wo, bo):
    import ml_dtypes
    bf = ml_dtypes.bfloat16
    KQ, KC, NI = QD // 128, CD // 128, INNER // 128
    Wqs = _sw(np.asarray(Wq).astype(bf), KQ)
    Wks = _sw(np.asarray(Wk).astype(bf), KC)
    Wvs = _sw(np.asarray(Wv).astype(bf), KC)
    Wos = _sw(np.asarray(Wo).astype(bf), NI)
    bo2 = np.ascontiguousarray(np.asarray(bo).astype(bf).reshape(1, QD))
    in_maps = []
    for c in range(8):
        b, q = divmod(c, 2)
        xT = np.asarray(x[b, q * NS:(q + 1) * NS, :]).astype(bf).T
        xTs = xT.reshape(KQ, 128, NS).transpose(1, 0, 2)  # [128, KQ, NS]
        cT = np.asarray(context[b]).astype(bf).T
        cTs = cT.reshape(KC, 128, M).transpose(1, 0, 2)   # [128, KC, M]
        im = {"Wq": Wqs, "Wk": Wks, "Wv": Wvs, "Wo": Wos, "bo": bo2}
        x0 = xTs[:, :, 0:512]
        im["x0a"] = np.ascontiguousarray(x0[:, 0:4].reshape(128, -1))
        im["x0b"] = np.ascontiguousarray(x0[:, 4:8].reshape(128, -1))
        im["x1"] = np.ascontiguousarray(
            xTs[:, :, 512:1024].reshape(128, -1))
        c0 = cTs[:, :, 0:512]
        im["c0a"] = np.ascontiguousarray(c0[:, 0:3].reshape(128, -1))
        im["c0b"] = np.ascontiguousarray(c0[:, 3:6].reshape(128, -1))
        for i in range(1, 4):
            im[f"ctx{i}"] = np.ascontiguousarray(
                cTs[:, :, i * 512:(i + 1) * 512].reshape(128, -1))
        in_maps.append(im)
    return in_maps


def kernel(x, context, Wq, Wk, Wv, Wo, bo, _trace=False):
    from concourse.bass_utils import run_bass_kernel_spmd

    nc = _get_nc()
    in_maps = _shard_inputs(x, context, Wq, Wk, Wv, Wo, bo)
    res = run_bass_kernel_spmd(nc, in_maps, core_ids=list(range(8)),
                               trace=_trace)
    out = np.empty((B, N, QD), np.float32)
    for c in range(8):
        b, q = divmod(c, 2)
        out[b, q * NS:(q + 1) * NS, :] = res.results[c]["out"].astype(
            np.float32)
    if _trace:
        kernel._last_result = res
    return out
